# revision 22
# baseline (speedup 1.0000x reference)
"""Routed MoE classifier head for Trainium2 (8 NeuronCores, SPMD).

The reference computes all 8 experts densely and selects; here each sample is
routed to exactly one expert.  On the host we gather samples by expert
(expert e -> core e), pad to a common S, and pre-transpose x so the
contraction dim D lands on SBUF partitions.  Each core runs a dense 2-layer
MLP (768 -> relu 384 -> 8) over its expert's samples:

  layer 1:  h^T = relu(W1^T x^T + b1)   accumulated over d-blocks per h-block
  layer 2:  y^T = W2^T h^T + b2

MODE selects the matmul operand precision for layer 1:
  "bf16"  - x/W1 in bfloat16 (1 col/cycle).  18 PE cycles/sample.
  "fp8x3" - x and W1 split into fp8e4 hi+lo halves; compute
            xh*Wh + xh*Wl + xl*Wh with DoubleRow matmuls (256-deep
            contraction at 0.5 cyc/row): 13.5 PE cycles/sample at the
            same DMA bytes as bf16 (hi+lo = 2B/elem).
Layer 2 and the PSUM accumulation stay f32/bf16; relu is fused into the
PSUM->SBUF eviction on the Scalar engine.  Output y^T [8, S] is scattered
back on the host.
"""

import numpy as np

import concourse.bass as bass
import concourse.mybir as mybir
from concourse.tile import TileContext
from concourse.bass_utils import run_bass_kernel_spmd

P = 128
D = 768
H = 384
C = 8
E = 8
NCORES = 8
DBLK = D // P  # 6
HBLK = H // P  # 3
CHUNK = 512  # compute chunk (one PSUM bank of fp32)
XGRAN = 1024  # x DMA granularity (samples per load), chunk-aligned
YGRAN = 2048  # y DMA granularity (samples per store)
WARMMM = 28  # PE p-state pre-warm dummy matmuls (64 cols each)

MODE = "bf16"  # "bf16" or "fp8x3"
W1SCALE = 64.0  # fp8 mode: W1 pre-scaled so its hi/lo split clears the
# e4m3 subnormal floor; folded back via the relu's input scale

BF16 = mybir.dt.bfloat16
F8 = mybir.dt.float8e4
NP_BF16 = mybir.dt.np(BF16)
NP_F8 = mybir.dt.np(F8)

_program_cache = {}
last_results = None  # BassKernelResults of the most recent run (for test harness)


def _split_excess_waits(nc, max_waits=1):
    """The walrus build in this container only encodes one sem-wait per
    instruction; hoist extra waits onto NOPs inserted just before."""
    for blk in nc.main_func.blocks:
        insts = blk.instructions
        i = 0
        while i < len(insts):
            inst = insts[i]
            si = getattr(inst, "sync_info", None)
            if si is not None and si.on_wait and len(si.on_wait) > max_waits:
                waits = list(si.on_wait)
                extra, keep = waits[:-max_waits], waits[-max_waits:]
                nops = []
                for j in range(0, len(extra), max_waits):
                    nops.append(
                        mybir.InstNoOp(
                            name=f"{inst.name}-wsplit{j}",
                            engine=inst.engine,
                            bass_nofuse=True,
                            sync_info=mybir.SyncInfo(
                                on_wait=extra[j : j + max_waits], on_update=[]
                            ),
                        )
                    )
                inst.sync_info = mybir.SyncInfo(on_wait=keep, on_update=si.on_update)
                for k, nop in enumerate(nops):
                    nc.register_instruction(nop, overwrite=True)
                    insts.insert(i + k, nop)
                i += len(nops)
            i += 1
    return nc


def _spans(total, lead, gran):
    """[(off, n), ...] covering `total`: leading spans from `lead`, then
    `gran`-sized spans (last one smaller)."""
    spans = []
    off = 0
    k = 0
    while off < total:
        n = min(lead[k] if k < len(lead) else gran, total - off)
        spans.append((off, n))
        off += n
        k += 1
    return spans


def _build_program(S, mode):
    f32 = mybir.dt.float32
    relu = mybir.ActivationFunctionType.Relu
    add = mybir.AluOpType.add
    fp8 = mode == "fp8x3"
    dr = mybir.MatmulPerfMode.DoubleRow if fp8 else None
    l1_scale = 1.0 / W1SCALE if fp8 else 1.0

    nc = bass.Bass(enable_partition_id=False)
    if fp8:
        xh = nc.dram_tensor("xh", [P, DBLK, S], F8, kind="ExternalInput")
        xl = nc.dram_tensor("xl", [P, DBLK, S], F8, kind="ExternalInput")
        w1h = nc.dram_tensor("w1h", [P, DBLK, H], F8, kind="ExternalInput")
        w1l = nc.dram_tensor("w1l", [P, DBLK, H], F8, kind="ExternalInput")
    else:
        xh = nc.dram_tensor("xh", [P, DBLK, S], BF16, kind="ExternalInput")
        w1h = nc.dram_tensor("w1h", [P, DBLK, H], BF16, kind="ExternalInput")
    w2t = nc.dram_tensor("w2t", [P, HBLK, C], BF16, kind="ExternalInput")
    # b1 (3 cols, per h-block) and b2 (1 col, rows 0..7) packed
    bt = nc.dram_tensor("bt", [P, HBLK + 1], f32, kind="ExternalInput")
    yt = nc.dram_tensor("yt", [C, S], f32, kind="ExternalOutput")

    x_spans = _spans(S, [], XGRAN)

    with TileContext(nc) as tc:
        with (
            tc.tile_pool(name="const", bufs=1) as cpool,
            tc.tile_pool(name="xin", bufs=3) as xpool,
            tc.tile_pool(name="hbuf", bufs=3) as hpool,
            tc.tile_pool(name="yout", bufs=2) as ypool,
            tc.tile_pool(name="psum1", bufs=6, space="PSUM") as pp1,
            tc.tile_pool(name="psum2", bufs=2, space="PSUM") as pp2,
        ):
            wdt = F8 if fp8 else BF16
            w1h_t = cpool.tile([P, DBLK, H], wdt)
            if fp8:
                w1l_t = cpool.tile([P, DBLK, H], wdt)
                # first-needed pieces (k-pair 0) land first, on separate
                # HWDGE queues so they stream in parallel
                nc.sync.dma_start(w1h_t[:, 0:2, :], w1h[:, 0:2, :])
                nc.gpsimd.dma_start(w1l_t[:, 0:2, :], w1l[:, 0:2, :])
                nc.scalar.dma_start(w1h_t[:, 2:, :], w1h[:, 2:, :])
                nc.sync.dma_start(w1l_t[:, 2:, :], w1l[:, 2:, :])
            w2_t = cpool.tile([P, HBLK, C], BF16)
            b_t = cpool.tile([P, HBLK + 1], f32)


            span_tiles = {}

            def load_x(span_idx):
                off, n = x_spans[span_idx]
                xh_t = xpool.tile([P, DBLK, XGRAN], wdt, name="xh_t")
                xl_t = xpool.tile([P, DBLK, XGRAN], wdt, name="xl_t") if fp8 else None
                if span_idx == 0:
                    # Startup choreography.  Three HWDGE queues (sync,
                    # scalar, gpsimd) with ~1.1us inter-entry latency each:
                    # order every queue so each piece lands just before the
                    # d-block-outer first chunk consumes it.
                    if fp8:
                        for t in range(DBLK // 2):
                            s = slice(2 * t, 2 * t + 2)
                            nc.sync.dma_start(xh_t[:, s, :n], xh[:, s, off : off + n])
                            nc.gpsimd.dma_start(xl_t[:, s, :n], xl[:, s, off : off + n])
                    else:
                        a = min(n, CHUNK)
                        nc.sync.dma_start(xh_t[:, 0:2, :a], xh[:, 0:2, off : off + a])
                        nc.scalar.dma_start(xh_t[:, 2:4, :a], xh[:, 2:4, off : off + a])
                        nc.gpsimd.dma_start(w1h_t[:, 0:4, :], w1h[:, 0:4, :])
                        nc.sync.dma_start(xh_t[:, 4:6, :a], xh[:, 4:6, off : off + a])
                        nc.scalar.dma_start(w1h_t[:, 4:6, :], w1h[:, 4:6, :])
                        nc.gpsimd.dma_start(b_t[:], bt[:])
                        if n > a:
                            nc.sync.dma_start(
                                xh_t[:, 0:2, a:n], xh[:, 0:2, off + a : off + n]
                            )
                            nc.scalar.dma_start(
                                xh_t[:, 2:4, a:n], xh[:, 2:4, off + a : off + n]
                            )
                            nc.sync.dma_start(
                                xh_t[:, 4:6, a:n], xh[:, 4:6, off + a : off + n]
                            )
                        nc.scalar.dma_start(w2_t[:], w2t[:])
                elif span_idx == 1 and not fp8:
                    # gpsimd's queue is free after b; sync still drains the
                    # span-0 pieces
                    nc.gpsimd.dma_start(xh_t[:, :, :n], xh[:, :, off : off + n])
                else:
                    nc.sync.dma_start(xh_t[:, :, :n], xh[:, :, off : off + n])
                    if fp8:
                        nc.gpsimd.dma_start(xl_t[:, :, :n], xl[:, :, off : off + n])
                span_tiles[span_idx] = (xh_t, xl_t)

            y_tile = None  # current [C, YGRAN] output staging tile
            y_base = 0
            y_seq = [0]  # alternate store queues so tail round-trips overlap

            def emit_l2(pend):
                # layer 2 for an already-relu'd chunk: y^T = W2^T h^T + b2.
                # The bias add runs on the Scalar engine (Identity+bias) so
                # the first matmul's h-ready and ps2-reuse deps land on the
                # same Activation semaphore and merge into one wait -- a
                # second wait would become a NOP that bubbles the PE.
                nonlocal y_tile, y_base
                h_t, off, n = pend
                ps2 = pp2.tile([C, CHUNK], f32, name="ps2")
                for hb in range(HBLK):
                    nc.tensor.matmul(
                        ps2[:, :n],
                        w2_t[:, hb, :],
                        h_t[:, hb, :n],
                        start=(hb == 0),
                        stop=(hb == HBLK - 1),
                    )
                if y_tile is None:
                    y_tile = ypool.tile([C, YGRAN], f32, name="y_t")
                    y_base = off
                lo = off - y_base
                nc.scalar.activation(
                    y_tile[:, lo : lo + n],
                    ps2[:, :n],
                    mybir.ActivationFunctionType.Identity,
                    bias=b_t[:C, HBLK : HBLK + 1],
                )
                if lo + n + CHUNK > YGRAN or off + n >= S:
                    eng = (nc.sync, nc.gpsimd)[y_seq[0] % 2]
                    y_seq[0] += 1
                    eng.dma_start(yt[:, y_base : y_base + lo + n], y_tile[:, : lo + n])
                    y_tile = None

            def l1_matmuls(ps, hb, xh_t, xl_t, o, n, outer_first, outer_last):
                """all layer-1 matmuls accumulating h-block hb into ps."""
                if fp8:
                    k = 0
                    for t in range(DBLK // 2):
                        s = slice(2 * t, 2 * t + 2)
                        for xx, ww in (
                            (xh_t, w1h_t),
                            (xh_t, w1l_t),
                            (xl_t, w1h_t),
                        ):
                            nc.tensor.matmul(
                                ps[:, :n],
                                ww[:, s, hb * P : (hb + 1) * P],
                                xx[:, s, o : o + n],
                                start=(k == 0),
                                stop=(k == 3 * (DBLK // 2) - 1),
                                perf_mode=dr,
                            )
                            k += 1
                else:
                    for db in range(DBLK):
                        nc.tensor.matmul(
                            ps[:, :n],
                            w1h_t[:, db, hb * P : (hb + 1) * P],
                            xh_t[:, db, o : o + n],
                            start=(db == 0),
                            stop=(db == DBLK - 1),
                        )

            # Software pipeline: emit layer-2 of chunk k-1 between layer-1 of
            # chunk k and k+1 so the PE never waits on the ACT-relu epilogue.
            load_x(0)

            # Warm the ACT table during the startup DMA window so the
            # first real relu doesn't pay the ~1.5us table load.
            warm = cpool.tile([P, 1], f32)
            nc.vector.memset(warm[:], 0.0)
            nc.scalar.activation(warm[:], warm[:], relu, bias=0.0)

            # Pre-warm the PE p-state with dummy matmuls on a zeroed tile
            # while the first x/w DMAs are in flight: the tensor engine
            # ramps from 1.2GHz to full clock after ~3us of activity, so
            # burn that ramp on throwaway work instead of real chunks.
            warm_w = cpool.tile([P, 64], BF16)
            nc.vector.memset(warm_w[:], 0.0)
            ps_warm = pp2.tile([C, CHUNK], f32, name="ps2")
            for _ in range(WARMMM):
                nc.tensor.matmul(ps_warm[:, :64], warm_w[:, :C], warm_w[:, :64])
            # L2 batched per two chunks: each L1<->L2 transition reconfigures
            # the PE output tile group, so halve how often that happens.
            pending = []

            def flush_pending():
                for p in pending:
                    emit_l2(p)
                pending.clear()

            for si, (soff, sn) in enumerate(x_spans):
                xh_t, xl_t = span_tiles.pop(si)
                for o in range(0, sn, CHUNK):
                    n = min(CHUNK, sn - o)
                    h_t = hpool.tile([P, HBLK, CHUNK], BF16, name="h_t")
                    if si == 0:
                        # k-outer: consume each arriving x slice across all
                        # h-block accumulators immediately
                        pss = [
                            pp1.tile([P, CHUNK], f32, name="ps")
                            for _ in range(HBLK)
                        ]
                        if fp8:
                            k = 0
                            for t in range(DBLK // 2):
                                s = slice(2 * t, 2 * t + 2)
                                for xx, ww in (
                                    (xh_t, w1h_t),
                                    (xh_t, w1l_t),
                                    (xl_t, w1h_t),
                                ):
                                    for hb in range(HBLK):
                                        nc.tensor.matmul(
                                            pss[hb][:, :n],
                                            ww[:, s, hb * P : (hb + 1) * P],
                                            xx[:, s, o : o + n],
                                            start=(k == 0),
                                            stop=(k == 3 * (DBLK // 2) - 1),
                                            perf_mode=dr,
                                        )
                                    k += 1
                        else:
                            for db in range(DBLK):
                                for hb in range(HBLK):
                                    nc.tensor.matmul(
                                        pss[hb][:, :n],
                                        w1h_t[:, db, hb * P : (hb + 1) * P],
                                        xh_t[:, db, o : o + n],
                                        start=(db == 0),
                                        stop=(db == DBLK - 1),
                                    )
                        for hb in range(HBLK):
                            nc.scalar.activation(
                                h_t[:, hb, :n], pss[hb][:, :n], relu,
                                bias=b_t[:, hb : hb + 1], scale=l1_scale,
                            )
                    else:
                        for hb in range(HBLK):
                            ps = pp1.tile([P, CHUNK], f32, name="ps")
                            l1_matmuls(ps, hb, xh_t, xl_t, o, n, True, True)
                            nc.scalar.activation(
                                h_t[:, hb, :n], ps[:, :n], relu,
                                bias=b_t[:, hb : hb + 1], scale=l1_scale,
                            )
                    if o == 0 and si + 1 < len(x_spans):
                        load_x(si + 1)
                    if len(pending) >= 2:
                        flush_pending()
                    pending.append((h_t, soff + o, n))
            flush_pending()

    return _split_excess_waits(nc)


def kernel(x, W1, b1, W2, b2, question_types):
    global last_results
    x = np.ascontiguousarray(np.asarray(x, dtype=np.float32))
    W1 = np.asarray(W1, dtype=np.float32)
    b1 = np.asarray(b1, dtype=np.float32)
    W2 = np.asarray(W2, dtype=np.float32)
    b2 = np.asarray(b2, dtype=np.float32)
    qt = np.asarray(question_types)
    N = x.shape[0]
    fp8 = MODE == "fp8x3"

    idx = [np.nonzero(qt == e)[0] for e in range(E)]
    counts = [len(i) for i in idx]
    S = max(int(np.ceil(max(counts) / 16) * 16), 2 * CHUNK)

    nc = _program_cache.get((S, MODE))
    if nc is None:
        nc = _build_program(S, MODE)
        _program_cache[(S, MODE)] = nc

    # cast once on the full tensors, then gather/pack per expert
    if fp8:
        xh_full = x.astype(NP_F8)
        xl_full = (x - xh_full.astype(np.float32)).astype(NP_F8)
        W1s = W1 * np.float32(W1SCALE)
        w1h_full = W1s.astype(NP_F8)
        w1l_full = (W1s - w1h_full.astype(np.float32)).astype(NP_F8)
    else:
        xh_full = x.astype(NP_BF16)
        w1h_full = W1.astype(NP_BF16)
    w2_full = W2.astype(NP_BF16)

    def pack_x(xe, cnt):
        # [cnt, D] -> [P, DBLK, S]
        xp = np.zeros((S, D), xe.dtype)
        xp[:cnt] = xe
        return np.ascontiguousarray(xp.T.reshape(DBLK, P, S).transpose(1, 0, 2))

    in_maps = []
    for e in range(E):
        cnt = counts[e]
        m = {"xh": pack_x(xh_full[idx[e]], cnt)}
        if fp8:
            m["xl"] = pack_x(xl_full[idx[e]], cnt)
            m["w1l"] = np.ascontiguousarray(
                w1l_full[e].reshape(DBLK, P, H).transpose(1, 0, 2)
            )
        m["w1h"] = np.ascontiguousarray(
            w1h_full[e].reshape(DBLK, P, H).transpose(1, 0, 2)
        )
        m["w2t"] = np.ascontiguousarray(
            w2_full[e].reshape(HBLK, P, C).transpose(1, 0, 2)
        )
        bt = np.zeros((P, HBLK + 1), np.float32)
        bt[:, :HBLK] = b1[e].reshape(HBLK, P).T
        bt[:C, HBLK] = b2[e]
        m["bt"] = bt
        in_maps.append(m)

    r = run_bass_kernel_spmd(nc, in_maps, list(range(NCORES)))
    last_results = r

    out = np.zeros((N, C), np.float32)
    for e in range(E):
        out[idx[e]] = r.results[e]["yt"][:, : counts[e]].T
    return out


# revision 24
# speedup vs baseline: 1.0007x; 1.0007x over previous
"""Routed MoE classifier head for Trainium2 (8 NeuronCores, SPMD).

The reference computes all 8 experts densely and selects; here each sample is
routed to exactly one expert.  On the host we gather samples by expert
(expert e -> core e), pad to a common S, and pre-transpose x so the
contraction dim D lands on SBUF partitions.  Each core runs a dense 2-layer
MLP (768 -> relu 384 -> 8) over its expert's samples:

  layer 1:  h^T = relu(W1^T x^T + b1)   accumulated over d-blocks per h-block
  layer 2:  y^T = W2^T h^T + b2

MODE selects the matmul operand precision for layer 1:
  "bf16"  - x/W1 in bfloat16 (1 col/cycle).  18 PE cycles/sample.
  "fp8x3" - x and W1 split into fp8e4 hi+lo halves; compute
            xh*Wh + xh*Wl + xl*Wh with DoubleRow matmuls (256-deep
            contraction at 0.5 cyc/row): 13.5 PE cycles/sample at the
            same DMA bytes as bf16 (hi+lo = 2B/elem).
Layer 2 and the PSUM accumulation stay f32/bf16; relu is fused into the
PSUM->SBUF eviction on the Scalar engine.  Output y^T [8, S] is scattered
back on the host.
"""

import numpy as np

import concourse.bass as bass
import concourse.mybir as mybir
from concourse.tile import TileContext
from concourse.bass_utils import run_bass_kernel_spmd

P = 128
D = 768
H = 384
C = 8
E = 8
NCORES = 8
DBLK = D // P  # 6
HBLK = H // P  # 3
CHUNK = 512  # compute chunk (one PSUM bank of fp32)
XGRAN = 1024  # x DMA granularity (samples per load), chunk-aligned
YGRAN = 2048  # y DMA granularity (samples per store)
WARMMM = 45  # PE p-state pre-warm dummy matmuls (64 cols each)

MODE = "bf16"  # "bf16" or "fp8x3"
W1SCALE = 64.0  # fp8 mode: W1 pre-scaled so its hi/lo split clears the
# e4m3 subnormal floor; folded back via the relu's input scale

BF16 = mybir.dt.bfloat16
F8 = mybir.dt.float8e4
NP_BF16 = mybir.dt.np(BF16)
NP_F8 = mybir.dt.np(F8)

_program_cache = {}
last_results = None  # BassKernelResults of the most recent run (for test harness)


def _split_excess_waits(nc, max_waits=1):
    """The walrus build in this container only encodes one sem-wait per
    instruction; hoist extra waits onto NOPs inserted just before."""
    for blk in nc.main_func.blocks:
        insts = blk.instructions
        i = 0
        while i < len(insts):
            inst = insts[i]
            si = getattr(inst, "sync_info", None)
            if si is not None and si.on_wait and len(si.on_wait) > max_waits:
                waits = list(si.on_wait)
                extra, keep = waits[:-max_waits], waits[-max_waits:]
                nops = []
                for j in range(0, len(extra), max_waits):
                    nops.append(
                        mybir.InstNoOp(
                            name=f"{inst.name}-wsplit{j}",
                            engine=inst.engine,
                            bass_nofuse=True,
                            sync_info=mybir.SyncInfo(
                                on_wait=extra[j : j + max_waits], on_update=[]
                            ),
                        )
                    )
                inst.sync_info = mybir.SyncInfo(on_wait=keep, on_update=si.on_update)
                for k, nop in enumerate(nops):
                    nc.register_instruction(nop, overwrite=True)
                    insts.insert(i + k, nop)
                i += len(nops)
            i += 1
    return nc


def _spans(total, lead, gran):
    """[(off, n), ...] covering `total`: leading spans from `lead`, then
    `gran`-sized spans (last one smaller)."""
    spans = []
    off = 0
    k = 0
    while off < total:
        n = min(lead[k] if k < len(lead) else gran, total - off)
        spans.append((off, n))
        off += n
        k += 1
    return spans


def _build_program(S, mode):
    f32 = mybir.dt.float32
    relu = mybir.ActivationFunctionType.Relu
    add = mybir.AluOpType.add
    fp8 = mode == "fp8x3"
    dr = mybir.MatmulPerfMode.DoubleRow if fp8 else None
    l1_scale = 1.0 / W1SCALE if fp8 else 1.0

    nc = bass.Bass(enable_partition_id=False)
    if fp8:
        xh = nc.dram_tensor("xh", [P, DBLK, S], F8, kind="ExternalInput")
        xl = nc.dram_tensor("xl", [P, DBLK, S], F8, kind="ExternalInput")
        w1h = nc.dram_tensor("w1h", [P, DBLK, H], F8, kind="ExternalInput")
        w1l = nc.dram_tensor("w1l", [P, DBLK, H], F8, kind="ExternalInput")
    else:
        xh = nc.dram_tensor("xh", [P, DBLK, S], BF16, kind="ExternalInput")
        w1h = nc.dram_tensor("w1h", [P, DBLK, H], BF16, kind="ExternalInput")
    w2t = nc.dram_tensor("w2t", [P, HBLK, C], BF16, kind="ExternalInput")
    # b1 (3 cols, per h-block) and b2 (1 col, rows 0..7) packed
    bt = nc.dram_tensor("bt", [P, HBLK + 1], f32, kind="ExternalInput")
    yt = nc.dram_tensor("yt", [C, S], f32, kind="ExternalOutput")

    x_spans = _spans(S, [], XGRAN)

    with TileContext(nc) as tc:
        with (
            tc.tile_pool(name="const", bufs=1) as cpool,
            tc.tile_pool(name="xin", bufs=3) as xpool,
            tc.tile_pool(name="hbuf", bufs=3) as hpool,
            tc.tile_pool(name="yout", bufs=2) as ypool,
            tc.tile_pool(name="psum1", bufs=6, space="PSUM") as pp1,
            tc.tile_pool(name="psum2", bufs=2, space="PSUM") as pp2,
        ):
            wdt = F8 if fp8 else BF16
            w1h_t = cpool.tile([P, DBLK, H], wdt)
            if fp8:
                w1l_t = cpool.tile([P, DBLK, H], wdt)
                # first-needed pieces (k-pair 0) land first, on separate
                # HWDGE queues so they stream in parallel
                nc.sync.dma_start(w1h_t[:, 0:2, :], w1h[:, 0:2, :])
                nc.gpsimd.dma_start(w1l_t[:, 0:2, :], w1l[:, 0:2, :])
                nc.scalar.dma_start(w1h_t[:, 2:, :], w1h[:, 2:, :])
                nc.sync.dma_start(w1l_t[:, 2:, :], w1l[:, 2:, :])
            w2_t = cpool.tile([P, HBLK, C], BF16)
            b_t = cpool.tile([P, HBLK + 1], f32)


            span_tiles = {}

            def load_x(span_idx):
                off, n = x_spans[span_idx]
                xh_t = xpool.tile([P, DBLK, XGRAN], wdt, name="xh_t")
                xl_t = xpool.tile([P, DBLK, XGRAN], wdt, name="xl_t") if fp8 else None
                if span_idx == 0:
                    # Startup choreography.  Three HWDGE queues (sync,
                    # scalar, gpsimd) with ~1.1us inter-entry latency each:
                    # order every queue so each piece lands just before the
                    # d-block-outer first chunk consumes it.
                    if fp8:
                        for t in range(DBLK // 2):
                            s = slice(2 * t, 2 * t + 2)
                            nc.sync.dma_start(xh_t[:, s, :n], xh[:, s, off : off + n])
                            nc.gpsimd.dma_start(xl_t[:, s, :n], xl[:, s, off : off + n])
                    else:
                        # sync's queue consistently starts earliest out of
                        # the preamble, so it carries the gating w1 block;
                        # x pieces lead scalar/gpsimd.
                        a = min(n, CHUNK)
                        nc.sync.dma_start(w1h_t[:, 0:4, :], w1h[:, 0:4, :])
                        nc.scalar.dma_start(xh_t[:, 0:2, :a], xh[:, 0:2, off : off + a])
                        nc.gpsimd.dma_start(xh_t[:, 2:4, :a], xh[:, 2:4, off : off + a])
                        nc.sync.dma_start(xh_t[:, 4:6, :a], xh[:, 4:6, off : off + a])
                        nc.scalar.dma_start(w1h_t[:, 4:6, :], w1h[:, 4:6, :])
                        nc.gpsimd.dma_start(b_t[:], bt[:])
                        if n > a:
                            nc.sync.dma_start(
                                xh_t[:, 0:2, a:n], xh[:, 0:2, off + a : off + n]
                            )
                            nc.scalar.dma_start(
                                xh_t[:, 2:4, a:n], xh[:, 2:4, off + a : off + n]
                            )
                            nc.gpsimd.dma_start(
                                xh_t[:, 4:6, a:n], xh[:, 4:6, off + a : off + n]
                            )
                        nc.scalar.dma_start(w2_t[:], w2t[:])
                elif span_idx == 1 and not fp8:
                    # still inside the startup scramble: keep span 1 in
                    # per-queue pieces so no single queue serializes it
                    nc.sync.dma_start(xh_t[:, 0:2, :n], xh[:, 0:2, off : off + n])
                    nc.scalar.dma_start(xh_t[:, 2:4, :n], xh[:, 2:4, off : off + n])
                    nc.gpsimd.dma_start(xh_t[:, 4:6, :n], xh[:, 4:6, off : off + n])
                else:
                    nc.sync.dma_start(xh_t[:, :, :n], xh[:, :, off : off + n])
                    if fp8:
                        nc.gpsimd.dma_start(xl_t[:, :, :n], xl[:, :, off : off + n])
                span_tiles[span_idx] = (xh_t, xl_t)

            y_tile = None  # current [C, YGRAN] output staging tile
            y_base = 0
            y_seq = [0]  # alternate store queues so tail round-trips overlap

            def emit_l2(pend):
                # layer 2 for an already-relu'd chunk: y^T = W2^T h^T + b2.
                # The bias add runs on the Scalar engine (Identity+bias) so
                # the first matmul's h-ready and ps2-reuse deps land on the
                # same Activation semaphore and merge into one wait -- a
                # second wait would become a NOP that bubbles the PE.
                nonlocal y_tile, y_base
                h_t, off, n = pend
                ps2 = pp2.tile([C, CHUNK], f32, name="ps2")
                for hb in range(HBLK):
                    nc.tensor.matmul(
                        ps2[:, :n],
                        w2_t[:, hb, :],
                        h_t[:, hb, :n],
                        start=(hb == 0),
                        stop=(hb == HBLK - 1),
                    )
                if y_tile is None:
                    y_tile = ypool.tile([C, YGRAN], f32, name="y_t")
                    y_base = off
                lo = off - y_base
                nc.scalar.activation(
                    y_tile[:, lo : lo + n],
                    ps2[:, :n],
                    mybir.ActivationFunctionType.Identity,
                    bias=b_t[:C, HBLK : HBLK + 1],
                )
                if lo + n + CHUNK > YGRAN or off + n >= S:
                    eng = (nc.sync, nc.gpsimd)[y_seq[0] % 2]
                    y_seq[0] += 1
                    eng.dma_start(yt[:, y_base : y_base + lo + n], y_tile[:, : lo + n])
                    y_tile = None

            def l1_matmuls(ps, hb, xh_t, xl_t, o, n, outer_first, outer_last):
                """all layer-1 matmuls accumulating h-block hb into ps."""
                if fp8:
                    k = 0
                    for t in range(DBLK // 2):
                        s = slice(2 * t, 2 * t + 2)
                        for xx, ww in (
                            (xh_t, w1h_t),
                            (xh_t, w1l_t),
                            (xl_t, w1h_t),
                        ):
                            nc.tensor.matmul(
                                ps[:, :n],
                                ww[:, s, hb * P : (hb + 1) * P],
                                xx[:, s, o : o + n],
                                start=(k == 0),
                                stop=(k == 3 * (DBLK // 2) - 1),
                                perf_mode=dr,
                            )
                            k += 1
                else:
                    for db in range(DBLK):
                        nc.tensor.matmul(
                            ps[:, :n],
                            w1h_t[:, db, hb * P : (hb + 1) * P],
                            xh_t[:, db, o : o + n],
                            start=(db == 0),
                            stop=(db == DBLK - 1),
                        )

            # Software pipeline: emit layer-2 of chunk k-1 between layer-1 of
            # chunk k and k+1 so the PE never waits on the ACT-relu epilogue.
            load_x(0)

            # Warm the ACT table during the startup DMA window so the
            # first real relu doesn't pay the ~1.5us table load.
            warm = cpool.tile([P, 1], f32)
            nc.vector.memset(warm[:], 0.0)
            nc.scalar.activation(warm[:], warm[:], relu, bias=0.0)

            # Pre-warm the PE p-state with dummy matmuls on a zeroed tile
            # while the first x/w DMAs are in flight: the tensor engine
            # ramps from 1.2GHz to full clock after ~3us of activity, so
            # burn that ramp on throwaway work instead of real chunks.
            warm_w = cpool.tile([P, 64], BF16)
            nc.vector.memset(warm_w[:], 0.0)
            ps_warm = pp2.tile([C, CHUNK], f32, name="ps2")
            for _ in range(WARMMM):
                nc.tensor.matmul(ps_warm[:, :64], warm_w[:, :C], warm_w[:, :64])
            # L2 batched per two chunks: each L1<->L2 transition reconfigures
            # the PE output tile group, so halve how often that happens.
            pending = []

            def flush_pending():
                for p in pending:
                    emit_l2(p)
                pending.clear()

            for si, (soff, sn) in enumerate(x_spans):
                xh_t, xl_t = span_tiles.pop(si)
                for o in range(0, sn, CHUNK):
                    n = min(CHUNK, sn - o)
                    h_t = hpool.tile([P, HBLK, CHUNK], BF16, name="h_t")
                    if si == 0:
                        # k-outer: consume each arriving x slice across all
                        # h-block accumulators immediately
                        pss = [
                            pp1.tile([P, CHUNK], f32, name="ps")
                            for _ in range(HBLK)
                        ]
                        if fp8:
                            k = 0
                            for t in range(DBLK // 2):
                                s = slice(2 * t, 2 * t + 2)
                                for xx, ww in (
                                    (xh_t, w1h_t),
                                    (xh_t, w1l_t),
                                    (xl_t, w1h_t),
                                ):
                                    for hb in range(HBLK):
                                        nc.tensor.matmul(
                                            pss[hb][:, :n],
                                            ww[:, s, hb * P : (hb + 1) * P],
                                            xx[:, s, o : o + n],
                                            start=(k == 0),
                                            stop=(k == 3 * (DBLK // 2) - 1),
                                            perf_mode=dr,
                                        )
                                    k += 1
                        else:
                            for db in range(DBLK):
                                for hb in range(HBLK):
                                    nc.tensor.matmul(
                                        pss[hb][:, :n],
                                        w1h_t[:, db, hb * P : (hb + 1) * P],
                                        xh_t[:, db, o : o + n],
                                        start=(db == 0),
                                        stop=(db == DBLK - 1),
                                    )
                        for hb in range(HBLK):
                            nc.scalar.activation(
                                h_t[:, hb, :n], pss[hb][:, :n], relu,
                                bias=b_t[:, hb : hb + 1], scale=l1_scale,
                            )
                    else:
                        for hb in range(HBLK):
                            ps = pp1.tile([P, CHUNK], f32, name="ps")
                            l1_matmuls(ps, hb, xh_t, xl_t, o, n, True, True)
                            nc.scalar.activation(
                                h_t[:, hb, :n], ps[:, :n], relu,
                                bias=b_t[:, hb : hb + 1], scale=l1_scale,
                            )
                    if o == 0 and si + 1 < len(x_spans):
                        load_x(si + 1)
                    if len(pending) >= 2:
                        flush_pending()
                    pending.append((h_t, soff + o, n))
            flush_pending()

    return _split_excess_waits(nc)


def kernel(x, W1, b1, W2, b2, question_types):
    global last_results
    x = np.ascontiguousarray(np.asarray(x, dtype=np.float32))
    W1 = np.asarray(W1, dtype=np.float32)
    b1 = np.asarray(b1, dtype=np.float32)
    W2 = np.asarray(W2, dtype=np.float32)
    b2 = np.asarray(b2, dtype=np.float32)
    qt = np.asarray(question_types)
    N = x.shape[0]
    fp8 = MODE == "fp8x3"

    idx = [np.nonzero(qt == e)[0] for e in range(E)]
    counts = [len(i) for i in idx]
    S = max(int(np.ceil(max(counts) / 16) * 16), 2 * CHUNK)

    nc = _program_cache.get((S, MODE))
    if nc is None:
        nc = _build_program(S, MODE)
        _program_cache[(S, MODE)] = nc

    # cast once on the full tensors, then gather/pack per expert
    if fp8:
        xh_full = x.astype(NP_F8)
        xl_full = (x - xh_full.astype(np.float32)).astype(NP_F8)
        W1s = W1 * np.float32(W1SCALE)
        w1h_full = W1s.astype(NP_F8)
        w1l_full = (W1s - w1h_full.astype(np.float32)).astype(NP_F8)
    else:
        xh_full = x.astype(NP_BF16)
        w1h_full = W1.astype(NP_BF16)
    w2_full = W2.astype(NP_BF16)

    def pack_x(xe, cnt):
        # [cnt, D] -> [P, DBLK, S]
        xp = np.zeros((S, D), xe.dtype)
        xp[:cnt] = xe
        return np.ascontiguousarray(xp.T.reshape(DBLK, P, S).transpose(1, 0, 2))

    in_maps = []
    for e in range(E):
        cnt = counts[e]
        m = {"xh": pack_x(xh_full[idx[e]], cnt)}
        if fp8:
            m["xl"] = pack_x(xl_full[idx[e]], cnt)
            m["w1l"] = np.ascontiguousarray(
                w1l_full[e].reshape(DBLK, P, H).transpose(1, 0, 2)
            )
        m["w1h"] = np.ascontiguousarray(
            w1h_full[e].reshape(DBLK, P, H).transpose(1, 0, 2)
        )
        m["w2t"] = np.ascontiguousarray(
            w2_full[e].reshape(HBLK, P, C).transpose(1, 0, 2)
        )
        bt = np.zeros((P, HBLK + 1), np.float32)
        bt[:, :HBLK] = b1[e].reshape(HBLK, P).T
        bt[:C, HBLK] = b2[e]
        m["bt"] = bt
        in_maps.append(m)

    r = run_bass_kernel_spmd(nc, in_maps, list(range(NCORES)))
    last_results = r

    out = np.zeros((N, C), np.float32)
    for e in range(E):
        out[idx[e]] = r.results[e]["yt"][:, : counts[e]].T
    return out


# revision 25
# speedup vs baseline: 1.0061x; 1.0054x over previous
"""Routed MoE classifier head for Trainium2 (8 NeuronCores, SPMD).

The reference computes all 8 experts densely and selects; here each sample is
routed to exactly one expert.  On the host we gather samples by expert
(expert e -> core e), pad to a common S, and pre-transpose x so the
contraction dim D lands on SBUF partitions.  Each core runs a dense 2-layer
MLP (768 -> relu 384 -> 8) over its expert's samples:

  layer 1:  h^T = relu(W1^T x^T + b1)   accumulated over d-blocks per h-block
  layer 2:  y^T = W2^T h^T + b2

MODE selects the matmul operand precision for layer 1:
  "bf16"  - x/W1 in bfloat16 (1 col/cycle).  18 PE cycles/sample.
  "fp8x3" - x and W1 split into fp8e4 hi+lo halves; compute
            xh*Wh + xh*Wl + xl*Wh with DoubleRow matmuls (256-deep
            contraction at 0.5 cyc/row): 13.5 PE cycles/sample at the
            same DMA bytes as bf16 (hi+lo = 2B/elem).
Layer 2 and the PSUM accumulation stay f32/bf16; relu is fused into the
PSUM->SBUF eviction on the Scalar engine.  Output y^T [8, S] is scattered
back on the host.
"""

import numpy as np

import concourse.bass as bass
import concourse.mybir as mybir
from concourse.tile import TileContext
from concourse.bass_utils import run_bass_kernel_spmd

P = 128
D = 768
H = 384
C = 8
E = 8
NCORES = 8
DBLK = D // P  # 6
HBLK = H // P  # 3
CHUNK = 512  # compute chunk (one PSUM bank of fp32)
XGRAN = 1024  # x DMA granularity (samples per load), chunk-aligned
YGRAN = 2048  # y DMA granularity (samples per store)
WARMMM = 45  # PE p-state pre-warm dummy matmuls (64 cols each)

MODE = "bf16"  # "bf16" or "fp8x3"
W1SCALE = 64.0  # fp8 mode: W1 pre-scaled so its hi/lo split clears the
# e4m3 subnormal floor; folded back via the relu's input scale

BF16 = mybir.dt.bfloat16
F8 = mybir.dt.float8e4
NP_BF16 = mybir.dt.np(BF16)
NP_F8 = mybir.dt.np(F8)

_program_cache = {}
last_results = None  # BassKernelResults of the most recent run (for test harness)


def _split_excess_waits(nc, max_waits=1):
    """The walrus build in this container only encodes one sem-wait per
    instruction; hoist extra waits onto NOPs inserted just before."""
    for blk in nc.main_func.blocks:
        insts = blk.instructions
        i = 0
        while i < len(insts):
            inst = insts[i]
            si = getattr(inst, "sync_info", None)
            if si is not None and si.on_wait and len(si.on_wait) > max_waits:
                waits = list(si.on_wait)
                extra, keep = waits[:-max_waits], waits[-max_waits:]
                nops = []
                for j in range(0, len(extra), max_waits):
                    nops.append(
                        mybir.InstNoOp(
                            name=f"{inst.name}-wsplit{j}",
                            engine=inst.engine,
                            bass_nofuse=True,
                            sync_info=mybir.SyncInfo(
                                on_wait=extra[j : j + max_waits], on_update=[]
                            ),
                        )
                    )
                inst.sync_info = mybir.SyncInfo(on_wait=keep, on_update=si.on_update)
                for k, nop in enumerate(nops):
                    nc.register_instruction(nop, overwrite=True)
                    insts.insert(i + k, nop)
                i += len(nops)
            i += 1
    return nc


def _spans(total, lead, gran):
    """[(off, n), ...] covering `total`: leading spans from `lead`, then
    `gran`-sized spans (last one smaller)."""
    spans = []
    off = 0
    k = 0
    while off < total:
        n = min(lead[k] if k < len(lead) else gran, total - off)
        spans.append((off, n))
        off += n
        k += 1
    return spans


def _build_program(S, mode):
    f32 = mybir.dt.float32
    relu = mybir.ActivationFunctionType.Relu
    add = mybir.AluOpType.add
    fp8 = mode == "fp8x3"
    dr = mybir.MatmulPerfMode.DoubleRow if fp8 else None
    l1_scale = 1.0 / W1SCALE if fp8 else 1.0

    nc = bass.Bass(enable_partition_id=False)
    if fp8:
        xh = nc.dram_tensor("xh", [P, DBLK, S], F8, kind="ExternalInput")
        xl = nc.dram_tensor("xl", [P, DBLK, S], F8, kind="ExternalInput")
        w1h = nc.dram_tensor("w1h", [P, DBLK, H], F8, kind="ExternalInput")
        w1l = nc.dram_tensor("w1l", [P, DBLK, H], F8, kind="ExternalInput")
    else:
        xh = nc.dram_tensor("xh", [P, DBLK, S], BF16, kind="ExternalInput")
        w1h = nc.dram_tensor("w1h", [P, DBLK, H], BF16, kind="ExternalInput")
    w2t = nc.dram_tensor("w2t", [P, HBLK, P], BF16, kind="ExternalInput")
    # b1 (3 cols, per h-block) and b2 (1 col, rows 0..7) packed
    bt = nc.dram_tensor("bt", [P, HBLK + 1], f32, kind="ExternalInput")
    yt = nc.dram_tensor("yt", [C, S], f32, kind="ExternalOutput")

    x_spans = _spans(S, [], XGRAN)

    with TileContext(nc) as tc:
        with (
            tc.tile_pool(name="const", bufs=1) as cpool,
            tc.tile_pool(name="xin", bufs=3) as xpool,
            tc.tile_pool(name="hbuf", bufs=3) as hpool,
            tc.tile_pool(name="yout", bufs=2) as ypool,
            tc.tile_pool(name="psum1", bufs=6, space="PSUM") as pp1,
            tc.tile_pool(name="psum2", bufs=2, space="PSUM") as pp2,
        ):
            wdt = F8 if fp8 else BF16
            w1h_t = cpool.tile([P, DBLK, H], wdt)
            if fp8:
                w1l_t = cpool.tile([P, DBLK, H], wdt)
                # first-needed pieces (k-pair 0) land first, on separate
                # HWDGE queues so they stream in parallel
                nc.sync.dma_start(w1h_t[:, 0:2, :], w1h[:, 0:2, :])
                nc.gpsimd.dma_start(w1l_t[:, 0:2, :], w1l[:, 0:2, :])
                nc.scalar.dma_start(w1h_t[:, 2:, :], w1h[:, 2:, :])
                nc.sync.dma_start(w1l_t[:, 2:, :], w1l[:, 2:, :])
            w2_t = cpool.tile([P, HBLK, P], BF16)
            b_t = cpool.tile([P, HBLK + 1], f32)


            span_tiles = {}

            def load_x(span_idx):
                off, n = x_spans[span_idx]
                xh_t = xpool.tile([P, DBLK, XGRAN], wdt, name="xh_t")
                xl_t = xpool.tile([P, DBLK, XGRAN], wdt, name="xl_t") if fp8 else None
                if span_idx == 0:
                    # Startup choreography.  Three HWDGE queues (sync,
                    # scalar, gpsimd) with ~1.1us inter-entry latency each:
                    # order every queue so each piece lands just before the
                    # d-block-outer first chunk consumes it.
                    if fp8:
                        for t in range(DBLK // 2):
                            s = slice(2 * t, 2 * t + 2)
                            nc.sync.dma_start(xh_t[:, s, :n], xh[:, s, off : off + n])
                            nc.gpsimd.dma_start(xl_t[:, s, :n], xl[:, s, off : off + n])
                    else:
                        # sync's queue consistently starts earliest out of
                        # the preamble, so it carries the gating w1 block;
                        # x pieces lead scalar/gpsimd.
                        a = min(n, CHUNK)
                        nc.sync.dma_start(w1h_t[:, 0:4, :], w1h[:, 0:4, :])
                        nc.scalar.dma_start(xh_t[:, 0:2, :a], xh[:, 0:2, off : off + a])
                        nc.gpsimd.dma_start(xh_t[:, 2:4, :a], xh[:, 2:4, off : off + a])
                        nc.sync.dma_start(xh_t[:, 4:6, :a], xh[:, 4:6, off : off + a])
                        nc.scalar.dma_start(w1h_t[:, 4:6, :], w1h[:, 4:6, :])
                        nc.gpsimd.dma_start(b_t[:], bt[:])
                        if n > a:
                            nc.sync.dma_start(
                                xh_t[:, 0:2, a:n], xh[:, 0:2, off + a : off + n]
                            )
                            nc.scalar.dma_start(
                                xh_t[:, 2:4, a:n], xh[:, 2:4, off + a : off + n]
                            )
                            nc.gpsimd.dma_start(
                                xh_t[:, 4:6, a:n], xh[:, 4:6, off + a : off + n]
                            )
                        nc.scalar.dma_start(w2_t[:], w2t[:])
                elif span_idx == 1 and not fp8:
                    # still inside the startup scramble: keep span 1 in
                    # per-queue pieces so no single queue serializes it
                    nc.sync.dma_start(xh_t[:, 0:2, :n], xh[:, 0:2, off : off + n])
                    nc.scalar.dma_start(xh_t[:, 2:4, :n], xh[:, 2:4, off : off + n])
                    nc.gpsimd.dma_start(xh_t[:, 4:6, :n], xh[:, 4:6, off : off + n])
                else:
                    nc.sync.dma_start(xh_t[:, :, :n], xh[:, :, off : off + n])
                    if fp8:
                        nc.gpsimd.dma_start(xl_t[:, :, :n], xl[:, :, off : off + n])
                span_tiles[span_idx] = (xh_t, xl_t)

            y_tile = None  # current [C, YGRAN] output staging tile
            y_base = 0
            y_seq = [0]  # alternate store queues so tail round-trips overlap

            def emit_l2(pend):
                # layer 2 for an already-relu'd chunk: y^T = W2^T h^T + b2.
                # The bias add runs on the Scalar engine (Identity+bias) so
                # the first matmul's h-ready and ps2-reuse deps land on the
                # same Activation semaphore and merge into one wait -- a
                # second wait would become a NOP that bubbles the PE.
                nonlocal y_tile, y_base
                h_t, off, n = pend
                # W2 is zero-padded to 128 output columns so layer-2
                # matmuls keep the same (128,128) PE tile config as layer 1
                # (a config switch costs ~95ns of pipeline bubble).
                ps2 = pp2.tile([P, CHUNK], f32, name="ps2")
                for hb in range(HBLK):
                    nc.tensor.matmul(
                        ps2[:, :n],
                        w2_t[:, hb, :],
                        h_t[:, hb, :n],
                        start=(hb == 0),
                        stop=(hb == HBLK - 1),
                    )
                if y_tile is None:
                    y_tile = ypool.tile([C, YGRAN], f32, name="y_t")
                    y_base = off
                lo = off - y_base
                nc.scalar.activation(
                    y_tile[:, lo : lo + n],
                    ps2[:C, :n],
                    mybir.ActivationFunctionType.Identity,
                    bias=b_t[:C, HBLK : HBLK + 1],
                )
                if lo + n + CHUNK > YGRAN or off + n >= S:
                    eng = (nc.sync, nc.gpsimd)[y_seq[0] % 2]
                    y_seq[0] += 1
                    eng.dma_start(yt[:, y_base : y_base + lo + n], y_tile[:, : lo + n])
                    y_tile = None

            def l1_matmuls(ps, hb, xh_t, xl_t, o, n, outer_first, outer_last):
                """all layer-1 matmuls accumulating h-block hb into ps."""
                if fp8:
                    k = 0
                    for t in range(DBLK // 2):
                        s = slice(2 * t, 2 * t + 2)
                        for xx, ww in (
                            (xh_t, w1h_t),
                            (xh_t, w1l_t),
                            (xl_t, w1h_t),
                        ):
                            nc.tensor.matmul(
                                ps[:, :n],
                                ww[:, s, hb * P : (hb + 1) * P],
                                xx[:, s, o : o + n],
                                start=(k == 0),
                                stop=(k == 3 * (DBLK // 2) - 1),
                                perf_mode=dr,
                            )
                            k += 1
                else:
                    for db in range(DBLK):
                        nc.tensor.matmul(
                            ps[:, :n],
                            w1h_t[:, db, hb * P : (hb + 1) * P],
                            xh_t[:, db, o : o + n],
                            start=(db == 0),
                            stop=(db == DBLK - 1),
                        )

            # Software pipeline: emit layer-2 of chunk k-1 between layer-1 of
            # chunk k and k+1 so the PE never waits on the ACT-relu epilogue.
            load_x(0)

            # Warm the ACT table during the startup DMA window so the
            # first real relu doesn't pay the ~1.5us table load.
            warm = cpool.tile([P, 1], f32)
            nc.vector.memset(warm[:], 0.0)
            nc.scalar.activation(warm[:], warm[:], relu, bias=0.0)

            # Pre-warm the PE p-state with dummy matmuls on a zeroed tile
            # while the first x/w DMAs are in flight: the tensor engine
            # ramps from 1.2GHz to full clock after ~3us of activity, so
            # burn that ramp on throwaway work instead of real chunks.
            warm_w = cpool.tile([P, 64], BF16)
            warm_w2 = cpool.tile([P, P], BF16)
            nc.vector.memset(warm_w[:], 0.0)
            nc.vector.memset(warm_w2[:], 0.0)
            ps_warm = pp2.tile([P, CHUNK], f32, name="ps2")
            for _ in range(WARMMM):
                nc.tensor.matmul(ps_warm[:, :64], warm_w2[:], warm_w[:, :64])
            # L2 batched per two chunks: each L1<->L2 transition reconfigures
            # the PE output tile group, so halve how often that happens.
            pending = []

            def flush_pending():
                for p in pending:
                    emit_l2(p)
                pending.clear()

            for si, (soff, sn) in enumerate(x_spans):
                xh_t, xl_t = span_tiles.pop(si)
                for o in range(0, sn, CHUNK):
                    n = min(CHUNK, sn - o)
                    h_t = hpool.tile([P, HBLK, CHUNK], BF16, name="h_t")
                    if si == 0:
                        # k-outer: consume each arriving x slice across all
                        # h-block accumulators immediately
                        pss = [
                            pp1.tile([P, CHUNK], f32, name="ps")
                            for _ in range(HBLK)
                        ]
                        if fp8:
                            k = 0
                            for t in range(DBLK // 2):
                                s = slice(2 * t, 2 * t + 2)
                                for xx, ww in (
                                    (xh_t, w1h_t),
                                    (xh_t, w1l_t),
                                    (xl_t, w1h_t),
                                ):
                                    for hb in range(HBLK):
                                        nc.tensor.matmul(
                                            pss[hb][:, :n],
                                            ww[:, s, hb * P : (hb + 1) * P],
                                            xx[:, s, o : o + n],
                                            start=(k == 0),
                                            stop=(k == 3 * (DBLK // 2) - 1),
                                            perf_mode=dr,
                                        )
                                    k += 1
                        else:
                            for db in range(DBLK):
                                for hb in range(HBLK):
                                    nc.tensor.matmul(
                                        pss[hb][:, :n],
                                        w1h_t[:, db, hb * P : (hb + 1) * P],
                                        xh_t[:, db, o : o + n],
                                        start=(db == 0),
                                        stop=(db == DBLK - 1),
                                    )
                        for hb in range(HBLK):
                            nc.scalar.activation(
                                h_t[:, hb, :n], pss[hb][:, :n], relu,
                                bias=b_t[:, hb : hb + 1], scale=l1_scale,
                            )
                    else:
                        for hb in range(HBLK):
                            ps = pp1.tile([P, CHUNK], f32, name="ps")
                            l1_matmuls(ps, hb, xh_t, xl_t, o, n, True, True)
                            nc.scalar.activation(
                                h_t[:, hb, :n], ps[:, :n], relu,
                                bias=b_t[:, hb : hb + 1], scale=l1_scale,
                            )
                    if o == 0 and si + 1 < len(x_spans):
                        load_x(si + 1)
                    if len(pending) >= 2:
                        flush_pending()
                    pending.append((h_t, soff + o, n))
            flush_pending()

    return _split_excess_waits(nc)


def kernel(x, W1, b1, W2, b2, question_types):
    global last_results
    x = np.ascontiguousarray(np.asarray(x, dtype=np.float32))
    W1 = np.asarray(W1, dtype=np.float32)
    b1 = np.asarray(b1, dtype=np.float32)
    W2 = np.asarray(W2, dtype=np.float32)
    b2 = np.asarray(b2, dtype=np.float32)
    qt = np.asarray(question_types)
    N = x.shape[0]
    fp8 = MODE == "fp8x3"

    idx = [np.nonzero(qt == e)[0] for e in range(E)]
    counts = [len(i) for i in idx]
    S = max(int(np.ceil(max(counts) / 16) * 16), 2 * CHUNK)

    nc = _program_cache.get((S, MODE))
    if nc is None:
        nc = _build_program(S, MODE)
        _program_cache[(S, MODE)] = nc

    # cast once on the full tensors, then gather/pack per expert
    if fp8:
        xh_full = x.astype(NP_F8)
        xl_full = (x - xh_full.astype(np.float32)).astype(NP_F8)
        W1s = W1 * np.float32(W1SCALE)
        w1h_full = W1s.astype(NP_F8)
        w1l_full = (W1s - w1h_full.astype(np.float32)).astype(NP_F8)
    else:
        xh_full = x.astype(NP_BF16)
        w1h_full = W1.astype(NP_BF16)
    w2_full = W2.astype(NP_BF16)

    def pack_x(xe, cnt):
        # [cnt, D] -> [P, DBLK, S]
        xp = np.zeros((S, D), xe.dtype)
        xp[:cnt] = xe
        return np.ascontiguousarray(xp.T.reshape(DBLK, P, S).transpose(1, 0, 2))

    in_maps = []
    for e in range(E):
        cnt = counts[e]
        m = {"xh": pack_x(xh_full[idx[e]], cnt)}
        if fp8:
            m["xl"] = pack_x(xl_full[idx[e]], cnt)
            m["w1l"] = np.ascontiguousarray(
                w1l_full[e].reshape(DBLK, P, H).transpose(1, 0, 2)
            )
        m["w1h"] = np.ascontiguousarray(
            w1h_full[e].reshape(DBLK, P, H).transpose(1, 0, 2)
        )
        w2p = np.zeros((HBLK, P, P), NP_BF16)
        w2p[:, :, :C] = w2_full[e].reshape(HBLK, P, C)
        m["w2t"] = np.ascontiguousarray(w2p.transpose(1, 0, 2))
        bt = np.zeros((P, HBLK + 1), np.float32)
        bt[:, :HBLK] = b1[e].reshape(HBLK, P).T
        bt[:C, HBLK] = b2[e]
        m["bt"] = bt
        in_maps.append(m)

    r = run_bass_kernel_spmd(nc, in_maps, list(range(NCORES)))
    last_results = r

    out = np.zeros((N, C), np.float32)
    for e in range(E):
        out[idx[e]] = r.results[e]["yt"][:, : counts[e]].T
    return out


# revision 28
# speedup vs baseline: 1.0168x; 1.0107x over previous
"""Routed MoE classifier head for Trainium2 (8 NeuronCores, SPMD).

The reference computes all 8 experts densely and selects; here each sample is
routed to exactly one expert.  On the host we gather samples by expert
(expert e -> core e), pad to a common S, and pre-transpose x so the
contraction dim D lands on SBUF partitions.  Each core runs a dense 2-layer
MLP (768 -> relu 384 -> 8) over its expert's samples:

  layer 1:  h^T = relu(W1^T x^T + b1)   accumulated over d-blocks per h-block
  layer 2:  y^T = W2^T h^T + b2

MODE selects the matmul operand precision for layer 1:
  "bf16"  - x/W1 in bfloat16 (1 col/cycle).  18 PE cycles/sample.
  "fp8x3" - x and W1 split into fp8e4 hi+lo halves; compute
            xh*Wh + xh*Wl + xl*Wh with DoubleRow matmuls (256-deep
            contraction at 0.5 cyc/row): 13.5 PE cycles/sample at the
            same DMA bytes as bf16 (hi+lo = 2B/elem).
Layer 2 and the PSUM accumulation stay f32/bf16; relu is fused into the
PSUM->SBUF eviction on the Scalar engine.  Output y^T [8, S] is scattered
back on the host.
"""

import numpy as np

import concourse.bass as bass
import concourse.mybir as mybir
from concourse.tile import TileContext
from concourse.bass_utils import run_bass_kernel_spmd

P = 128
D = 768
H = 384
C = 8
E = 8
NCORES = 8
DBLK = D // P  # 6
HBLK = H // P  # 3
CHUNK = 512  # compute chunk (one PSUM bank of fp32)
XGRAN = 1024  # x DMA granularity (samples per load), chunk-aligned
YGRAN = 2048  # y DMA granularity (samples per store)
WARMMM = 50  # PE p-state pre-warm dummy matmuls (64 cols each): bridge
# the whole DMA-gated startup window so the clock ramp carries into the
# first real chunk instead of resetting during an idle gap

MODE = "bf16"  # "bf16" or "fp8x3"
W1SCALE = 64.0  # fp8 mode: W1 pre-scaled so its hi/lo split clears the
# e4m3 subnormal floor; folded back via the relu's input scale

BF16 = mybir.dt.bfloat16
F8 = mybir.dt.float8e4
NP_BF16 = mybir.dt.np(BF16)
NP_F8 = mybir.dt.np(F8)

_program_cache = {}
last_results = None  # BassKernelResults of the most recent run (for test harness)


def _split_excess_waits(nc, max_waits=1):
    """The walrus build in this container only encodes one sem-wait per
    instruction; hoist extra waits onto NOPs inserted just before."""
    for blk in nc.main_func.blocks:
        insts = blk.instructions
        i = 0
        while i < len(insts):
            inst = insts[i]
            si = getattr(inst, "sync_info", None)
            if si is not None and si.on_wait and len(si.on_wait) > max_waits:
                waits = list(si.on_wait)
                extra, keep = waits[:-max_waits], waits[-max_waits:]
                nops = []
                for j in range(0, len(extra), max_waits):
                    nops.append(
                        mybir.InstNoOp(
                            name=f"{inst.name}-wsplit{j}",
                            engine=inst.engine,
                            bass_nofuse=True,
                            sync_info=mybir.SyncInfo(
                                on_wait=extra[j : j + max_waits], on_update=[]
                            ),
                        )
                    )
                inst.sync_info = mybir.SyncInfo(on_wait=keep, on_update=si.on_update)
                for k, nop in enumerate(nops):
                    nc.register_instruction(nop, overwrite=True)
                    insts.insert(i + k, nop)
                i += len(nops)
            i += 1
    return nc


def _spans(total, lead, gran):
    """[(off, n), ...] covering `total`: leading spans from `lead`, then
    `gran`-sized spans (last one smaller)."""
    spans = []
    off = 0
    k = 0
    while off < total:
        n = min(lead[k] if k < len(lead) else gran, total - off)
        spans.append((off, n))
        off += n
        k += 1
    return spans


def _build_program(S, mode):
    f32 = mybir.dt.float32
    relu = mybir.ActivationFunctionType.Relu
    add = mybir.AluOpType.add
    fp8 = mode == "fp8x3"
    dr = mybir.MatmulPerfMode.DoubleRow if fp8 else None
    l1_scale = 1.0 / W1SCALE if fp8 else 1.0

    nc = bass.Bass(enable_partition_id=False)
    if fp8:
        xh = nc.dram_tensor("xh", [P, DBLK, S], F8, kind="ExternalInput")
        xl = nc.dram_tensor("xl", [P, DBLK, S], F8, kind="ExternalInput")
        w1h = nc.dram_tensor("w1h", [P, DBLK, H], F8, kind="ExternalInput")
        w1l = nc.dram_tensor("w1l", [P, DBLK, H], F8, kind="ExternalInput")
    else:
        xh = nc.dram_tensor("xh", [P, DBLK, S], BF16, kind="ExternalInput")
        w1h = nc.dram_tensor("w1h", [P, DBLK, H], BF16, kind="ExternalInput")
    w2t = nc.dram_tensor("w2t", [P, HBLK, P], BF16, kind="ExternalInput")
    # b1 (3 cols, per h-block) and b2 (1 col, rows 0..7) packed
    bt = nc.dram_tensor("bt", [P, HBLK + 1], f32, kind="ExternalInput")
    yt = nc.dram_tensor("yt", [C, S], f32, kind="ExternalOutput")

    x_spans = _spans(S, [], XGRAN)

    with TileContext(nc) as tc:
        with (
            tc.tile_pool(name="const", bufs=1) as cpool,
            tc.tile_pool(name="xin", bufs=3) as xpool,
            tc.tile_pool(name="hbuf", bufs=3) as hpool,
            tc.tile_pool(name="yout", bufs=2) as ypool,
            tc.tile_pool(name="psum1", bufs=6, space="PSUM") as pp1,
            tc.tile_pool(name="psum2", bufs=2, space="PSUM") as pp2,
        ):
            wdt = F8 if fp8 else BF16
            w1h_t = cpool.tile([P, DBLK, H], wdt)
            if fp8:
                w1l_t = cpool.tile([P, DBLK, H], wdt)
                # first-needed pieces (k-pair 0) land first, on separate
                # HWDGE queues so they stream in parallel
                nc.sync.dma_start(w1h_t[:, 0:2, :], w1h[:, 0:2, :])
                nc.gpsimd.dma_start(w1l_t[:, 0:2, :], w1l[:, 0:2, :])
                nc.scalar.dma_start(w1h_t[:, 2:, :], w1h[:, 2:, :])
                nc.sync.dma_start(w1l_t[:, 2:, :], w1l[:, 2:, :])
            w2_t = cpool.tile([P, HBLK, P], BF16)
            b_t = cpool.tile([P, HBLK + 1], f32)


            span_tiles = {}

            def load_x(span_idx):
                off, n = x_spans[span_idx]
                xh_t = xpool.tile([P, DBLK, XGRAN], wdt, name="xh_t")
                xl_t = xpool.tile([P, DBLK, XGRAN], wdt, name="xl_t") if fp8 else None
                if span_idx == 0:
                    # Startup choreography.  Three HWDGE queues (sync,
                    # scalar, gpsimd) with ~1.1us inter-entry latency each:
                    # order every queue so each piece lands just before the
                    # d-block-outer first chunk consumes it.
                    if fp8:
                        for t in range(DBLK // 2):
                            s = slice(2 * t, 2 * t + 2)
                            nc.sync.dma_start(xh_t[:, s, :n], xh[:, s, off : off + n])
                            nc.gpsimd.dma_start(xl_t[:, s, :n], xl[:, s, off : off + n])
                    else:
                        # Pieces sized <=256KB and ordered per queue so each
                        # lands (at cold-start DMA rates) just before the
                        # d-block-outer first chunks consume it.
                        a = min(n, CHUNK)
                        nc.sync.dma_start(w1h_t[:, 0:2, :], w1h[:, 0:2, :])
                        nc.scalar.dma_start(xh_t[:, 0:2, :a], xh[:, 0:2, off : off + a])
                        nc.gpsimd.dma_start(xh_t[:, 2:4, :a], xh[:, 2:4, off : off + a])
                        nc.sync.dma_start(w1h_t[:, 2:4, :], w1h[:, 2:4, :])
                        nc.scalar.dma_start(w1h_t[:, 4:6, :], w1h[:, 4:6, :])
                        nc.gpsimd.dma_start(xh_t[:, 4:6, :a], xh[:, 4:6, off : off + a])
                        nc.sync.dma_start(b_t[:], bt[:])
                        if n > a:
                            nc.scalar.dma_start(
                                xh_t[:, 0:2, a:n], xh[:, 0:2, off + a : off + n]
                            )
                            nc.gpsimd.dma_start(
                                xh_t[:, 2:4, a:n], xh[:, 2:4, off + a : off + n]
                            )
                            nc.sync.dma_start(
                                xh_t[:, 4:6, a:n], xh[:, 4:6, off + a : off + n]
                            )
                        nc.scalar.dma_start(w2_t[:], w2t[:])
                elif span_idx == 1 and not fp8:
                    # still inside the startup scramble: keep span 1 in
                    # per-queue pieces so no single queue serializes it
                    nc.gpsimd.dma_start(xh_t[:, 0:2, :n], xh[:, 0:2, off : off + n])
                    nc.scalar.dma_start(xh_t[:, 2:4, :n], xh[:, 2:4, off : off + n])
                    nc.sync.dma_start(xh_t[:, 4:6, :n], xh[:, 4:6, off : off + n])
                else:
                    nc.sync.dma_start(xh_t[:, :, :n], xh[:, :, off : off + n])
                    if fp8:
                        nc.gpsimd.dma_start(xl_t[:, :, :n], xl[:, :, off : off + n])
                span_tiles[span_idx] = (xh_t, xl_t)

            y_tile = None  # current [C, YGRAN] output staging tile
            y_base = 0
            y_seq = [0]  # alternate store queues so tail round-trips overlap

            def emit_l2(pend):
                # layer 2 for an already-relu'd chunk: y^T = W2^T h^T + b2.
                # The bias add runs on the Scalar engine (Identity+bias) so
                # the first matmul's h-ready and ps2-reuse deps land on the
                # same Activation semaphore and merge into one wait -- a
                # second wait would become a NOP that bubbles the PE.
                nonlocal y_tile, y_base
                h_t, off, n = pend
                # W2 is zero-padded to 128 output columns so layer-2
                # matmuls keep the same (128,128) PE tile config as layer 1
                # (a config switch costs ~95ns of pipeline bubble).
                ps2 = pp2.tile([P, CHUNK], f32, name="ps2")
                for hb in range(HBLK):
                    nc.tensor.matmul(
                        ps2[:, :n],
                        w2_t[:, hb, :],
                        h_t[:, hb, :n],
                        start=(hb == 0),
                        stop=(hb == HBLK - 1),
                    )
                if y_tile is None:
                    y_tile = ypool.tile([C, YGRAN], f32, name="y_t")
                    y_base = off
                lo = off - y_base
                nc.scalar.activation(
                    y_tile[:, lo : lo + n],
                    ps2[:C, :n],
                    mybir.ActivationFunctionType.Identity,
                    bias=b_t[:C, HBLK : HBLK + 1],
                )
                if lo + n + CHUNK > YGRAN or off + n >= S:
                    eng = (nc.sync, nc.gpsimd)[y_seq[0] % 2]
                    y_seq[0] += 1
                    eng.dma_start(yt[:, y_base : y_base + lo + n], y_tile[:, : lo + n])
                    y_tile = None

            def l1_matmuls(ps, hb, xh_t, xl_t, o, n, outer_first, outer_last):
                """all layer-1 matmuls accumulating h-block hb into ps."""
                if fp8:
                    k = 0
                    for t in range(DBLK // 2):
                        s = slice(2 * t, 2 * t + 2)
                        for xx, ww in (
                            (xh_t, w1h_t),
                            (xh_t, w1l_t),
                            (xl_t, w1h_t),
                        ):
                            nc.tensor.matmul(
                                ps[:, :n],
                                ww[:, s, hb * P : (hb + 1) * P],
                                xx[:, s, o : o + n],
                                start=(k == 0),
                                stop=(k == 3 * (DBLK // 2) - 1),
                                perf_mode=dr,
                            )
                            k += 1
                else:
                    for db in range(DBLK):
                        nc.tensor.matmul(
                            ps[:, :n],
                            w1h_t[:, db, hb * P : (hb + 1) * P],
                            xh_t[:, db, o : o + n],
                            start=(db == 0),
                            stop=(db == DBLK - 1),
                        )

            # Software pipeline: emit layer-2 of chunk k-1 between layer-1 of
            # chunk k and k+1 so the PE never waits on the ACT-relu epilogue.
            load_x(0)

            # Warm the ACT table during the startup DMA window so the
            # first real relu doesn't pay the ~1.5us table load.
            warm = cpool.tile([P, 1], f32)
            nc.vector.memset(warm[:], 0.0)
            nc.scalar.activation(warm[:], warm[:], relu, bias=0.0)

            # Pre-warm the PE p-state with dummy matmuls on a zeroed tile
            # while the first x/w DMAs are in flight: the tensor engine
            # ramps from 1.2GHz to full clock after ~3us of activity, so
            # burn that ramp on throwaway work instead of real chunks.
            warm_w = cpool.tile([P, 64], BF16)
            warm_w2 = cpool.tile([P, P], BF16)
            nc.vector.memset(warm_w[:], 0.0)
            nc.vector.memset(warm_w2[:], 0.0)
            ps_warm = pp2.tile([P, CHUNK], f32, name="ps2")
            for _ in range(WARMMM):
                nc.tensor.matmul(ps_warm[:, :64], warm_w2[:], warm_w[:, :64])
            # L2 batched per two chunks: each L1<->L2 transition reconfigures
            # the PE output tile group, so halve how often that happens.
            pending = []

            def flush_pending():
                for p in pending:
                    emit_l2(p)
                pending.clear()

            for si, (soff, sn) in enumerate(x_spans):
                xh_t, xl_t = span_tiles.pop(si)
                for o in range(0, sn, CHUNK):
                    n = min(CHUNK, sn - o)
                    h_t = hpool.tile([P, HBLK, CHUNK], BF16, name="h_t")
                    if si == 0:
                        # k-outer: consume each arriving x slice across all
                        # h-block accumulators immediately
                        pss = [
                            pp1.tile([P, CHUNK], f32, name="ps")
                            for _ in range(HBLK)
                        ]
                        if fp8:
                            k = 0
                            for t in range(DBLK // 2):
                                s = slice(2 * t, 2 * t + 2)
                                for xx, ww in (
                                    (xh_t, w1h_t),
                                    (xh_t, w1l_t),
                                    (xl_t, w1h_t),
                                ):
                                    for hb in range(HBLK):
                                        nc.tensor.matmul(
                                            pss[hb][:, :n],
                                            ww[:, s, hb * P : (hb + 1) * P],
                                            xx[:, s, o : o + n],
                                            start=(k == 0),
                                            stop=(k == 3 * (DBLK // 2) - 1),
                                            perf_mode=dr,
                                        )
                                    k += 1
                        else:
                            for db in range(DBLK):
                                for hb in range(HBLK):
                                    nc.tensor.matmul(
                                        pss[hb][:, :n],
                                        w1h_t[:, db, hb * P : (hb + 1) * P],
                                        xh_t[:, db, o : o + n],
                                        start=(db == 0),
                                        stop=(db == DBLK - 1),
                                    )
                        for hb in range(HBLK):
                            nc.scalar.activation(
                                h_t[:, hb, :n], pss[hb][:, :n], relu,
                                bias=b_t[:, hb : hb + 1], scale=l1_scale,
                            )
                    else:
                        for hb in range(HBLK):
                            ps = pp1.tile([P, CHUNK], f32, name="ps")
                            l1_matmuls(ps, hb, xh_t, xl_t, o, n, True, True)
                            nc.scalar.activation(
                                h_t[:, hb, :n], ps[:, :n], relu,
                                bias=b_t[:, hb : hb + 1], scale=l1_scale,
                            )
                    if o == 0 and si + 1 < len(x_spans):
                        load_x(si + 1)
                    if len(pending) >= 2:
                        flush_pending()
                    pending.append((h_t, soff + o, n))
            flush_pending()

    return _split_excess_waits(nc)


def kernel(x, W1, b1, W2, b2, question_types):
    global last_results
    x = np.ascontiguousarray(np.asarray(x, dtype=np.float32))
    W1 = np.asarray(W1, dtype=np.float32)
    b1 = np.asarray(b1, dtype=np.float32)
    W2 = np.asarray(W2, dtype=np.float32)
    b2 = np.asarray(b2, dtype=np.float32)
    qt = np.asarray(question_types)
    N = x.shape[0]
    fp8 = MODE == "fp8x3"

    idx = [np.nonzero(qt == e)[0] for e in range(E)]
    counts = [len(i) for i in idx]
    S = max(int(np.ceil(max(counts) / 16) * 16), 2 * CHUNK)

    nc = _program_cache.get((S, MODE))
    if nc is None:
        nc = _build_program(S, MODE)
        _program_cache[(S, MODE)] = nc

    # cast once on the full tensors, then gather/pack per expert
    if fp8:
        xh_full = x.astype(NP_F8)
        xl_full = (x - xh_full.astype(np.float32)).astype(NP_F8)
        W1s = W1 * np.float32(W1SCALE)
        w1h_full = W1s.astype(NP_F8)
        w1l_full = (W1s - w1h_full.astype(np.float32)).astype(NP_F8)
    else:
        xh_full = x.astype(NP_BF16)
        w1h_full = W1.astype(NP_BF16)
    w2_full = W2.astype(NP_BF16)

    def pack_x(xe, cnt):
        # [cnt, D] -> [P, DBLK, S]
        xp = np.zeros((S, D), xe.dtype)
        xp[:cnt] = xe
        return np.ascontiguousarray(xp.T.reshape(DBLK, P, S).transpose(1, 0, 2))

    in_maps = []
    for e in range(E):
        cnt = counts[e]
        m = {"xh": pack_x(xh_full[idx[e]], cnt)}
        if fp8:
            m["xl"] = pack_x(xl_full[idx[e]], cnt)
            m["w1l"] = np.ascontiguousarray(
                w1l_full[e].reshape(DBLK, P, H).transpose(1, 0, 2)
            )
        m["w1h"] = np.ascontiguousarray(
            w1h_full[e].reshape(DBLK, P, H).transpose(1, 0, 2)
        )
        w2p = np.zeros((HBLK, P, P), NP_BF16)
        w2p[:, :, :C] = w2_full[e].reshape(HBLK, P, C)
        m["w2t"] = np.ascontiguousarray(w2p.transpose(1, 0, 2))
        bt = np.zeros((P, HBLK + 1), np.float32)
        bt[:, :HBLK] = b1[e].reshape(HBLK, P).T
        bt[:C, HBLK] = b2[e]
        m["bt"] = bt
        in_maps.append(m)

    r = run_bass_kernel_spmd(nc, in_maps, list(range(NCORES)))
    last_results = r

    out = np.zeros((N, C), np.float32)
    for e in range(E):
        out[idx[e]] = r.results[e]["yt"][:, : counts[e]].T
    return out
